# revision 4
# baseline (speedup 1.0000x reference)
"""Trainium2 Bass kernel for nn_ColRepeatCausalLinear.

Math: reference computes out = x @ W + bias with
    W[s, t] = v[t] * d^(t-s)  for t >= s, else 0,   d = clip(decay_value, 0.9, 1)
which factorizes as a decayed prefix scan along S:
    y[b, e, t] = d * y[b, e, t-1] + x[b, e, t]
    out[b, e, t] = v[t] * y[b, e, t] + bias[t]
i.e. O(B*E*S) work instead of the O(B*E*S^2) dense matmul.

Mapping: data-parallel over B across 8 NeuronCores (x[b] per core, params
replicated). Per core the kernel sits on the DMA wall: 8 MiB in + 8 MiB
out against a measured ~428 GB/s aggregate DMA fabric (16 engines), so
wall clock ~= (first-byte latency ~8.6us, fixed NEFF preamble + queue
arm) + 16.4 MiB / 428 GB/s + epilogue. Structure to stay at the wall:
  - every [128, 2*S] x-tile is loaded as four quarter-DMAs (64
    partitions x one S-column half, 8 KiB/descriptor) split across BOTH
    HWDGE queues, so each DVE scan gates on aggregate fabric progress
    (~2.3 us/quarter) instead of one queue's ~214 GB/s share;
  - all load triggers enqueue before any store trigger on both rings
    (an event-gated store descriptor never head-of-line blocks a load);
  - stores are likewise quartered and enqueued per-scan, keeping both
    rings supplied through the load->store transition;
  - v is host-cast to bf16 and broadcast across partitions with a K=1
    ones-matmul into PSUM (bf16 moving data runs the PE at 1 cyc/row vs
    4 for fp32, so vb is ready ~2 us after v lands, off the scan gate);
  - the scan+scale runs per e-row chunk on the Vector engine via a fused
    custom DVE op (cumsum * v in one pass, ~1 cyc/elem).

Hardcoded problem shapes: x (8, 1024, 2048) f32, weight (1, 2048),
bias (2048,), decay_value (1,).
"""

import numpy as np

import concourse.bacc as bacc
import concourse.mybir as mybir
from concourse.tile import TileContext
from concourse.bass_utils import run_bass_kernel_spmd

B, E, S = 8, 1024, 2048
P = 128
H = 64  # partition half for quarter-DMAs
N_CORES = 8
F32 = mybir.dt.float32
BF16 = mybir.dt.bfloat16

_cache = {}

# Fused custom DVE op: out[p,k] = (sum_{j<=k} x[p,j]) * v[p,k] — the whole
# d=1 kernel body in ONE Vector-engine instruction (the stock path needs a
# 2-cyc/elem TensorTensorScan plus a 1-cyc/elem tensor_mul). Registered at
# runtime into dve_ops.OPS; sha self-pinned since this op isn't in-tree.
_FUSED_OP = None
try:
    from concourse import dve_ops as _dops
    from concourse.dve_spec import AluOp as _AluOp, Spec as _Spec
    from concourse.dve_spec import Src0 as _Src0, Src1 as _Src1, scan as _scan
    from concourse.dve_spec import lower as _lower
    from concourse.dve_uop import DveOpSpec as _DveOpSpec

    _FUSED_NAME = "CUMSUM_VSCALE_ANT"
    if _FUSED_NAME in _dops._SUB_OPCODE_FOR_NAME:
        _FUSED_OP = next(o for o in _dops.OPS if o.name == _FUSED_NAME)
    else:
        _fspec = _Spec(body=_scan(_AluOp.ADD, _Src0) * _Src1)
        _row = _dops._CUSTOM_DVE_ROW_BASE + len(_dops.OPS)
        assert _row < 0x20
        _dops._SUB_OPCODE_FOR_NAME[_FUSED_NAME] = _row
        _sha = {}
        for _ver in ("v3", "v4"):
            try:
                _sha[_ver] = _DveOpSpec(
                    name=_FUSED_NAME,
                    opcode=_row,
                    uops=_lower(_fspec, ver=_ver),
                    rd1_en=_dops.has_src1(_fspec),
                ).sha(_ver)
            except Exception:
                pass
        _FUSED_OP = _dops.DveOp(_FUSED_NAME, _fspec, subdim=False, uops_sha=_sha)
        _dops.OPS.append(_FUSED_OP)
        _dops.CUSTOM_DVE_SPECS[_FUSED_NAME] = _fspec
except Exception:
    _FUSED_OP = None

R = 2  # e-rows per partition per tile
BANK = 512  # fp32 elems per PSUM bank


def _build(d: float, has_bias: bool):
    nc = bacc.Bacc(
        "TRN2",
        target_bir_lowering=False,
        debug=False,
        enable_asserts=False,
    )
    x = nc.dram_tensor("x", [E, S], F32, kind="ExternalInput").ap()
    v_dram = nc.dram_tensor("v", [1, S], BF16, kind="ExternalInput").ap()
    bias_dram = None
    if has_bias:
        bias_dram = nc.dram_tensor("bias", [1, S], F32, kind="ExternalInput").ap()
    out = nc.dram_tensor("out", [E, S], F32, kind="ExternalOutput").ap()

    n_tiles = E // (P * R)
    rows = P * R

    with TileContext(nc) as tc:
        with (
            tc.tile_pool(name="const", bufs=1) as cpool,
            tc.tile_pool(name="xs", bufs=n_tiles) as xpool,
            tc.tile_pool(name="ys", bufs=2) as ypool,
            tc.tile_pool(name="os", bufs=n_tiles) as opool,
            tc.tile_pool(name="ps", bufs=1, space="PSUM") as ppool,
        ):
            # v (4 KiB bf16) rides first on the SP ring so the PE broadcast
            # is ready ~2 us after the queue arms.
            vrow = cpool.tile([1, S], BF16)
            nc.sync.dma_start(out=vrow[:], in_=v_dram)
            if has_bias:
                brow = cpool.tile([1, S], F32)
                nc.scalar.dma_start(out=brow[:], in_=bias_dram)

            # x loads: per tile, four quarter-DMAs (partition half x column
            # half). Partition half 'a' (0:64) on SP, 'b' (64:128) on Act;
            # column half c0 before c1 so scan (i, c) gates on the two
            # queues' joint progress through quarter index 2i+c.
            xts = []
            for i in range(n_tiles):
                xt = xpool.tile([P, R * S], F32)
                src = x[i * rows : (i + 1) * rows, :].rearrange(
                    "(p b) s -> p (b s)", b=R
                )
                for c in range(R):
                    cs = slice(c * S, (c + 1) * S)
                    nc.sync.dma_start(out=xt[0:H, cs], in_=src[0:H, cs])
                    nc.scalar.dma_start(out=xt[H:P, cs], in_=src[H:P, cs])
                xts.append(xt)

            # Broadcast v across partitions with a K=1 matmul against a
            # ones row (out[p, t] = v[t]); bf16 moving data streams the PE
            # at 1 cyc/row. The Vector engine reads vb from PSUM.
            ones = cpool.tile([1, P], BF16)
            nc.vector.memset(ones[:], 1.0)
            vb = ppool.tile([P, S], F32)
            for n in range(S // BANK):
                nc.tensor.matmul(
                    vb[:, n * BANK : (n + 1) * BANK],
                    ones[:],
                    vrow[:, n * BANK : (n + 1) * BANK],
                    start=True,
                    stop=True,
                )
            if has_bias:
                onesf = cpool.tile([1, P], F32)
                nc.vector.memset(onesf[:], 1.0)
                bb = ppool.tile([P, S], F32)
                for n in range(S // BANK):
                    nc.tensor.matmul(
                        bb[:, n * BANK : (n + 1) * BANK],
                        onesf[:],
                        brow[:, n * BANK : (n + 1) * BANK],
                        start=True,
                        stop=True,
                    )
            if not (d == 1.0 and _FUSED_OP is not None):
                dtile = cpool.tile([P, 1], F32)
                nc.gpsimd.memset(dtile[:], d)

            for i in range(n_tiles):
                xt = xts[i]
                ot = opool.tile([P, R * S], F32)
                dst = out[i * rows : (i + 1) * rows, :].rearrange(
                    "(p b) s -> p (b s)", b=R
                )
                for c in range(R):
                    cs = slice(c * S, (c + 1) * S)
                    xc = xt[:, cs]
                    oc = ot[:, cs]
                    if d == 1.0 and _FUSED_OP is not None:
                        nc.vector._custom_dve(_FUSED_OP, out=oc, in0=xc, in1=vb[:])
                    else:
                        yt = ypool.tile([P, S], F32)
                        nc.vector.tensor_tensor_scan(
                            yt[:], dtile[:].broadcast_to([P, S]), xc,
                            0.0, mybir.AluOpType.mult, mybir.AluOpType.add,
                        )
                        nc.vector.tensor_mul(oc, yt[:], vb[:])
                    if has_bias:
                        nc.vector.tensor_add(oc, oc, bb[:])
                    # Store the two partition halves of this scanned column
                    # half immediately, one per queue. Ring order stays
                    # loads-then-stores on both rings.
                    nc.sync.dma_start(out=dst[0:H, cs], in_=ot[0:H, cs])
                    nc.scalar.dma_start(out=dst[H:P, cs], in_=ot[H:P, cs])
    nc.compile()
    return nc


def _run(x, weight, bias, decay_value, trace=False):
    x = np.asarray(x, dtype=np.float32)
    weight = np.asarray(weight, dtype=np.float32)
    bias = np.asarray(bias, dtype=np.float32)
    decay_value = np.asarray(decay_value)
    assert x.shape == (B, E, S), x.shape

    # DECAY_CONSTANT = 1.0 in the reference; exponent is (t - s) / 1.0.
    d = float(np.clip(np.float64(decay_value.reshape(-1)[0]), 0.9, 1.0))
    has_bias = bool(np.any(bias))

    key = (d, has_bias)
    if key not in _cache:
        _cache[key] = _build(d, has_bias)
    nc = _cache[key]

    import ml_dtypes

    vrow = np.ascontiguousarray(
        weight.reshape(1, S).astype(ml_dtypes.bfloat16)
    )
    in_maps = []
    for b in range(N_CORES):
        m = {"x": np.ascontiguousarray(x[b]), "v": vrow}
        if has_bias:
            m["bias"] = np.ascontiguousarray(bias.reshape(1, S), dtype=np.float32)
        in_maps.append(m)

    res = run_bass_kernel_spmd(
        nc, in_maps, core_ids=list(range(N_CORES)), trace=trace
    )
    out = np.stack([r["out"] for r in res.results], axis=0)
    return out, res


def kernel(x, weight, bias, decay_value):
    out, _ = _run(x, weight, bias, decay_value)
    return out


# revision 6
# speedup vs baseline: 1.2594x; 1.2594x over previous
"""Trainium2 Bass kernel for nn_ColRepeatCausalLinear.

Math: reference computes out = x @ W + bias with
    W[s, t] = v[t] * d^(t-s)  for t >= s, else 0,   d = clip(decay_value, 0.9, 1)
which factorizes as a decayed prefix scan along S:
    y[b, e, t] = d * y[b, e, t-1] + x[b, e, t]
    out[b, e, t] = v[t] * y[b, e, t] + bias[t]
i.e. O(B*E*S) work instead of the O(B*E*S^2) dense matmul.

Mapping: data-parallel over B across 8 NeuronCores (x[b] per core, params
replicated). Per core the kernel sits on the DMA wall: 8 MiB in + 8 MiB
out against a measured ~428 GB/s aggregate DMA fabric (16 engines), so
wall clock ~= (first-byte latency ~8.6us, fixed NEFF preamble + queue
arm) + 16.4 MiB / 428 GB/s + epilogue. Structure to stay at the wall:
  - every [128, 2*S] x-tile is loaded as four quarter-DMAs (64
    partitions x one S-column half, 8 KiB/descriptor) split across BOTH
    HWDGE queues, so each DVE scan gates on aggregate fabric progress
    (~2.3 us/quarter) instead of one queue's ~214 GB/s share;
  - all load triggers enqueue before any store trigger on both rings
    (an event-gated store descriptor never head-of-line blocks a load);
  - stores are likewise quartered and enqueued per-scan, keeping both
    rings supplied through the load->store transition;
  - v is host-cast to bf16 and broadcast across partitions with a K=1
    ones-matmul into PSUM (bf16 moving data runs the PE at 1 cyc/row vs
    4 for fp32, so vb is ready ~2 us after v lands, off the scan gate);
  - the scan+scale runs per e-row chunk on the Vector engine via a fused
    custom DVE op (cumsum * v in one pass, ~1 cyc/elem).

Hardcoded problem shapes: x (8, 1024, 2048) f32, weight (1, 2048),
bias (2048,), decay_value (1,).
"""

import numpy as np

import concourse.bacc as bacc
import concourse.mybir as mybir
from concourse.tile import TileContext
from concourse.bass_utils import run_bass_kernel_spmd

B, E, S = 8, 1024, 2048
P = 128
H = 64  # partition half for quarter-DMAs
N_CORES = 8
F32 = mybir.dt.float32
BF16 = mybir.dt.bfloat16

_cache = {}

# Fused custom DVE op: out[p,k] = (sum_{j<=k} x[p,j]) * v[p,k] — the whole
# d=1 kernel body in ONE Vector-engine instruction (the stock path needs a
# 2-cyc/elem TensorTensorScan plus a 1-cyc/elem tensor_mul). Registered at
# runtime into dve_ops.OPS; sha self-pinned since this op isn't in-tree.
_FUSED_OP = None
try:
    from concourse import dve_ops as _dops
    from concourse.dve_spec import AluOp as _AluOp, Spec as _Spec
    from concourse.dve_spec import Src0 as _Src0, Src1 as _Src1, scan as _scan
    from concourse.dve_spec import lower as _lower
    from concourse.dve_uop import DveOpSpec as _DveOpSpec

    _FUSED_NAME = "CUMSUM_VSCALE_ANT"
    if _FUSED_NAME in _dops._SUB_OPCODE_FOR_NAME:
        _FUSED_OP = next(o for o in _dops.OPS if o.name == _FUSED_NAME)
    else:
        _fspec = _Spec(body=_scan(_AluOp.ADD, _Src0) * _Src1)
        _row = _dops._CUSTOM_DVE_ROW_BASE + len(_dops.OPS)
        assert _row < 0x20
        _dops._SUB_OPCODE_FOR_NAME[_FUSED_NAME] = _row
        _sha = {}
        for _ver in ("v3", "v4"):
            try:
                _sha[_ver] = _DveOpSpec(
                    name=_FUSED_NAME,
                    opcode=_row,
                    uops=_lower(_fspec, ver=_ver),
                    rd1_en=_dops.has_src1(_fspec),
                ).sha(_ver)
            except Exception:
                pass
        _FUSED_OP = _dops.DveOp(_FUSED_NAME, _fspec, subdim=False, uops_sha=_sha)
        _dops.OPS.append(_FUSED_OP)
        _dops.CUSTOM_DVE_SPECS[_FUSED_NAME] = _fspec
except Exception:
    _FUSED_OP = None

R = 2  # e-rows per partition per tile
BANK = 512  # fp32 elems per PSUM bank


def _build(d: float, has_bias: bool):
    nc = bacc.Bacc(
        "TRN2",
        target_bir_lowering=False,
        debug=False,
        enable_asserts=False,
    )
    x = nc.dram_tensor("x", [E, S], F32, kind="ExternalInput").ap()
    v_dram = nc.dram_tensor("v", [1, S], BF16, kind="ExternalInput").ap()
    bias_dram = None
    if has_bias:
        bias_dram = nc.dram_tensor("bias", [1, S], F32, kind="ExternalInput").ap()
    out = nc.dram_tensor("out", [E, S], F32, kind="ExternalOutput").ap()

    n_tiles = E // (P * R)
    rows = P * R

    with TileContext(nc) as tc:
        with (
            tc.tile_pool(name="const", bufs=1) as cpool,
            tc.tile_pool(name="xs", bufs=n_tiles) as xpool,
            tc.tile_pool(name="ys", bufs=2) as ypool,
            tc.tile_pool(name="os", bufs=n_tiles) as opool,
            tc.tile_pool(name="ps", bufs=1, space="PSUM") as ppool,
        ):
            # v (4 KiB bf16) rides first on the SP ring so the PE broadcast
            # is ready ~2 us after the queue arms.
            vrow = cpool.tile([1, S], BF16)
            nc.sync.dma_start(out=vrow[:], in_=v_dram)
            if has_bias:
                brow = cpool.tile([1, S], F32)
                nc.scalar.dma_start(out=brow[:], in_=bias_dram)

            # x loads: one 1 MiB HWDGE group per [128, S] column half (the
            # HWDGE queue sustains only ~4 outstanding groups — finer
            # splits stall the issuing engine on the 5th trigger). All c0
            # halves ride the Act queue and all c1 halves the SP queue, so
            # the scan chain's gates (0,c0),(0,c1),(1,c0),... alternate
            # between the queues' ~4.7 us/group paces instead of
            # serializing on one queue.
            xts = []
            for i in range(n_tiles):
                xt = xpool.tile([P, R * S], F32)
                src = x[i * rows : (i + 1) * rows, :].rearrange(
                    "(p b) s -> p (b s)", b=R
                )
                nc.scalar.dma_start(out=xt[:, 0:S], in_=src[:, 0:S])
                nc.sync.dma_start(out=xt[:, S : 2 * S], in_=src[:, S : 2 * S])
                xts.append(xt)

            # Broadcast v across partitions with a K=1 matmul against a
            # ones row (out[p, t] = v[t]); bf16 moving data streams the PE
            # at 1 cyc/row. The Vector engine reads vb from PSUM.
            ones = cpool.tile([1, P], BF16)
            nc.vector.memset(ones[:], 1.0)
            vb = ppool.tile([P, S], F32)
            for n in range(S // BANK):
                nc.tensor.matmul(
                    vb[:, n * BANK : (n + 1) * BANK],
                    ones[:],
                    vrow[:, n * BANK : (n + 1) * BANK],
                    start=True,
                    stop=True,
                )
            if has_bias:
                onesf = cpool.tile([1, P], F32)
                nc.vector.memset(onesf[:], 1.0)
                bb = ppool.tile([P, S], F32)
                for n in range(S // BANK):
                    nc.tensor.matmul(
                        bb[:, n * BANK : (n + 1) * BANK],
                        onesf[:],
                        brow[:, n * BANK : (n + 1) * BANK],
                        start=True,
                        stop=True,
                    )
            if not (d == 1.0 and _FUSED_OP is not None):
                dtile = cpool.tile([P, 1], F32)
                nc.gpsimd.memset(dtile[:], d)

            for i in range(n_tiles):
                xt = xts[i]
                ot = opool.tile([P, R * S], F32)
                dst = out[i * rows : (i + 1) * rows, :].rearrange(
                    "(p b) s -> p (b s)", b=R
                )
                for c in range(R):
                    cs = slice(c * S, (c + 1) * S)
                    xc = xt[:, cs]
                    oc = ot[:, cs]
                    if d == 1.0 and _FUSED_OP is not None:
                        nc.vector._custom_dve(_FUSED_OP, out=oc, in0=xc, in1=vb[:])
                    else:
                        yt = ypool.tile([P, S], F32)
                        nc.vector.tensor_tensor_scan(
                            yt[:], dtile[:].broadcast_to([P, S]), xc,
                            0.0, mybir.AluOpType.mult, mybir.AluOpType.add,
                        )
                        nc.vector.tensor_mul(oc, yt[:], vb[:])
                    if has_bias:
                        nc.vector.tensor_add(oc, oc, bb[:])
                    # Stores alternate queues in scan order so each ring's
                    # store groups arm in ring order; the last tile's two
                    # stores are split across both queues so the tail
                    # drains at the combined rate.
                    k = i * R + c
                    if i == n_tiles - 1:
                        nc.sync.dma_start(out=dst[0:H, cs], in_=ot[0:H, cs])
                        nc.scalar.dma_start(out=dst[H:P, cs], in_=ot[H:P, cs])
                    else:
                        eng = nc.sync if k % 2 == 0 else nc.scalar
                        eng.dma_start(out=dst[:, cs], in_=ot[:, cs])
    nc.compile()
    return nc


def _run(x, weight, bias, decay_value, trace=False):
    x = np.asarray(x, dtype=np.float32)
    weight = np.asarray(weight, dtype=np.float32)
    bias = np.asarray(bias, dtype=np.float32)
    decay_value = np.asarray(decay_value)
    assert x.shape == (B, E, S), x.shape

    # DECAY_CONSTANT = 1.0 in the reference; exponent is (t - s) / 1.0.
    d = float(np.clip(np.float64(decay_value.reshape(-1)[0]), 0.9, 1.0))
    has_bias = bool(np.any(bias))

    key = (d, has_bias)
    if key not in _cache:
        _cache[key] = _build(d, has_bias)
    nc = _cache[key]

    import ml_dtypes

    vrow = np.ascontiguousarray(
        weight.reshape(1, S).astype(ml_dtypes.bfloat16)
    )
    in_maps = []
    for b in range(N_CORES):
        m = {"x": np.ascontiguousarray(x[b]), "v": vrow}
        if has_bias:
            m["bias"] = np.ascontiguousarray(bias.reshape(1, S), dtype=np.float32)
        in_maps.append(m)

    res = run_bass_kernel_spmd(
        nc, in_maps, core_ids=list(range(N_CORES)), trace=trace
    )
    out = np.stack([r["out"] for r in res.results], axis=0)
    return out, res


def kernel(x, weight, bias, decay_value):
    out, _ = _run(x, weight, bias, decay_value)
    return out


# revision 7
# speedup vs baseline: 1.4333x; 1.1381x over previous
"""Trainium2 Bass kernel for nn_ColRepeatCausalLinear.

Math: reference computes out = x @ W + bias with
    W[s, t] = v[t] * d^(t-s)  for t >= s, else 0,   d = clip(decay_value, 0.9, 1)
which factorizes as a decayed prefix scan along S:
    y[b, e, t] = d * y[b, e, t-1] + x[b, e, t]
    out[b, e, t] = v[t] * y[b, e, t] + bias[t]
i.e. O(B*E*S) work instead of the O(B*E*S^2) dense matmul.

Mapping: data-parallel over B across 8 NeuronCores (x[b] per core, params
replicated). Per core the kernel sits on the DMA wall: 8 MiB in + 8 MiB
out against a measured ~428 GB/s aggregate DMA fabric (16 engines), so
wall clock ~= (first-byte latency ~8.6us, fixed NEFF preamble + queue
arm) + 16.4 MiB / 428 GB/s + epilogue. Structure to stay at the wall:
  - every [128, 2*S] x-tile is loaded as four quarter-DMAs (64
    partitions x one S-column half, 8 KiB/descriptor) split across BOTH
    HWDGE queues, so each DVE scan gates on aggregate fabric progress
    (~2.3 us/quarter) instead of one queue's ~214 GB/s share;
  - all load triggers enqueue before any store trigger on both rings
    (an event-gated store descriptor never head-of-line blocks a load);
  - stores are likewise quartered and enqueued per-scan, keeping both
    rings supplied through the load->store transition;
  - v is host-cast to bf16 and broadcast across partitions with a K=1
    ones-matmul into PSUM (bf16 moving data runs the PE at 1 cyc/row vs
    4 for fp32, so vb is ready ~2 us after v lands, off the scan gate);
  - the scan+scale runs per e-row chunk on the Vector engine via a fused
    custom DVE op (cumsum * v in one pass, ~1 cyc/elem).

Hardcoded problem shapes: x (8, 1024, 2048) f32, weight (1, 2048),
bias (2048,), decay_value (1,).
"""

import numpy as np

import concourse.bacc as bacc
import concourse.mybir as mybir
from concourse.tile import TileContext
from concourse.bass_utils import run_bass_kernel_spmd

B, E, S = 8, 1024, 2048
P = 128
H = 64  # partition half for quarter-DMAs
N_CORES = 8
F32 = mybir.dt.float32
BF16 = mybir.dt.bfloat16

_cache = {}

# Fused custom DVE op: out[p,k] = (sum_{j<=k} x[p,j]) * v[p,k] — the whole
# d=1 kernel body in ONE Vector-engine instruction (the stock path needs a
# 2-cyc/elem TensorTensorScan plus a 1-cyc/elem tensor_mul). Registered at
# runtime into dve_ops.OPS; sha self-pinned since this op isn't in-tree.
_FUSED_OP = None
try:
    from concourse import dve_ops as _dops
    from concourse.dve_spec import AluOp as _AluOp, Spec as _Spec
    from concourse.dve_spec import Src0 as _Src0, Src1 as _Src1, scan as _scan
    from concourse.dve_spec import lower as _lower
    from concourse.dve_uop import DveOpSpec as _DveOpSpec

    _FUSED_NAME = "CUMSUM_VSCALE_ANT"
    if _FUSED_NAME in _dops._SUB_OPCODE_FOR_NAME:
        _FUSED_OP = next(o for o in _dops.OPS if o.name == _FUSED_NAME)
    else:
        _fspec = _Spec(body=_scan(_AluOp.ADD, _Src0) * _Src1)
        _row = _dops._CUSTOM_DVE_ROW_BASE + len(_dops.OPS)
        assert _row < 0x20
        _dops._SUB_OPCODE_FOR_NAME[_FUSED_NAME] = _row
        _sha = {}
        for _ver in ("v3", "v4"):
            try:
                _sha[_ver] = _DveOpSpec(
                    name=_FUSED_NAME,
                    opcode=_row,
                    uops=_lower(_fspec, ver=_ver),
                    rd1_en=_dops.has_src1(_fspec),
                ).sha(_ver)
            except Exception:
                pass
        _FUSED_OP = _dops.DveOp(_FUSED_NAME, _fspec, subdim=False, uops_sha=_sha)
        _dops.OPS.append(_FUSED_OP)
        _dops.CUSTOM_DVE_SPECS[_FUSED_NAME] = _fspec
except Exception:
    _FUSED_OP = None

R = 2  # e-rows per partition per tile
BANK = 512  # fp32 elems per PSUM bank


def _build(d: float, has_bias: bool):
    nc = bacc.Bacc(
        "TRN2",
        target_bir_lowering=False,
        debug=False,
        enable_asserts=False,
    )
    x = nc.dram_tensor("x", [E, S], F32, kind="ExternalInput").ap()
    v_dram = nc.dram_tensor("v", [1, S], BF16, kind="ExternalInput").ap()
    bias_dram = None
    if has_bias:
        bias_dram = nc.dram_tensor("bias", [1, S], F32, kind="ExternalInput").ap()
    out = nc.dram_tensor("out", [E, S], F32, kind="ExternalOutput").ap()

    n_tiles = E // (P * R)
    rows = P * R

    with TileContext(nc) as tc:
        with (
            tc.tile_pool(name="const", bufs=1) as cpool,
            tc.tile_pool(name="xs", bufs=n_tiles) as xpool,
            tc.tile_pool(name="ys", bufs=2) as ypool,
            tc.tile_pool(name="os", bufs=n_tiles) as opool,
            tc.tile_pool(name="ps", bufs=1, space="PSUM") as ppool,
        ):
            # v (4 KiB bf16) rides first on the SP ring so the PE broadcast
            # is ready ~2 us after the queue arms.
            vrow = cpool.tile([1, S], BF16)
            nc.sync.dma_start(out=vrow[:], in_=v_dram)
            if has_bias:
                brow = cpool.tile([1, S], F32)
                nc.scalar.dma_start(out=brow[:], in_=bias_dram)

            # x loads: one 1 MiB HWDGE group per [128, S] column half (the
            # HWDGE queue sustains only ~4 outstanding groups — finer
            # splits stall the issuing engine on the 5th trigger). All c0
            # halves ride the Act queue and all c1 halves the SP queue, so
            # the scan chain's gates (0,c0),(0,c1),(1,c0),... alternate
            # between the queues' ~4.7 us/group paces instead of
            # serializing on one queue.
            xts = []
            for i in range(n_tiles):
                xt = xpool.tile([P, R * S], F32)
                src = x[i * rows : (i + 1) * rows, :].rearrange(
                    "(p b) s -> p (b s)", b=R
                )
                nc.scalar.dma_start(out=xt[:, 0:S], in_=src[:, 0:S])
                nc.sync.dma_start(out=xt[:, S : 2 * S], in_=src[:, S : 2 * S])
                xts.append(xt)

            # Broadcast v across partitions with a K=1 matmul against a
            # ones row (out[p, t] = v[t]); bf16 moving data streams the PE
            # at 1 cyc/row. The Vector engine reads vb from PSUM.
            ones = cpool.tile([1, P], BF16)
            nc.vector.memset(ones[:], 1.0)
            vb = ppool.tile([P, S], F32)
            for n in range(S // BANK):
                nc.tensor.matmul(
                    vb[:, n * BANK : (n + 1) * BANK],
                    ones[:],
                    vrow[:, n * BANK : (n + 1) * BANK],
                    start=True,
                    stop=True,
                )
            if has_bias:
                onesf = cpool.tile([1, P], F32)
                nc.vector.memset(onesf[:], 1.0)
                bb = ppool.tile([P, S], F32)
                for n in range(S // BANK):
                    nc.tensor.matmul(
                        bb[:, n * BANK : (n + 1) * BANK],
                        onesf[:],
                        brow[:, n * BANK : (n + 1) * BANK],
                        start=True,
                        stop=True,
                    )
            if not (d == 1.0 and _FUSED_OP is not None):
                dtile = cpool.tile([P, 1], F32)
                nc.gpsimd.memset(dtile[:], d)

            for i in range(n_tiles):
                xt = xts[i]
                ot = opool.tile([P, R * S], F32)
                dst = out[i * rows : (i + 1) * rows, :].rearrange(
                    "(p b) s -> p (b s)", b=R
                )
                for c in range(R):
                    cs = slice(c * S, (c + 1) * S)
                    xc = xt[:, cs]
                    oc = ot[:, cs]
                    if d == 1.0 and _FUSED_OP is not None:
                        nc.vector._custom_dve(_FUSED_OP, out=oc, in0=xc, in1=vb[:])
                    else:
                        yt = ypool.tile([P, S], F32)
                        nc.vector.tensor_tensor_scan(
                            yt[:], dtile[:].broadcast_to([P, S]), xc,
                            0.0, mybir.AluOpType.mult, mybir.AluOpType.add,
                        )
                        nc.vector.tensor_mul(oc, yt[:], vb[:])
                    if has_bias:
                        nc.vector.tensor_add(oc, oc, bb[:])
                    # Store each scanned column half as one 1 MiB group on
                    # the queue opposite its load (c0 loads ride Act, so
                    # c0 stores ride SP): 4 MiB of loads + 4 MiB of stores
                    # per ring, loads always ahead of stores.
                    eng = nc.sync if c == 0 else nc.scalar
                    eng.dma_start(out=dst[:, cs], in_=ot[:, cs])
    nc.compile()
    return nc


def _run(x, weight, bias, decay_value, trace=False):
    x = np.asarray(x, dtype=np.float32)
    weight = np.asarray(weight, dtype=np.float32)
    bias = np.asarray(bias, dtype=np.float32)
    decay_value = np.asarray(decay_value)
    assert x.shape == (B, E, S), x.shape

    # DECAY_CONSTANT = 1.0 in the reference; exponent is (t - s) / 1.0.
    d = float(np.clip(np.float64(decay_value.reshape(-1)[0]), 0.9, 1.0))
    has_bias = bool(np.any(bias))

    key = (d, has_bias)
    if key not in _cache:
        _cache[key] = _build(d, has_bias)
    nc = _cache[key]

    import ml_dtypes

    vrow = np.ascontiguousarray(
        weight.reshape(1, S).astype(ml_dtypes.bfloat16)
    )
    in_maps = []
    for b in range(N_CORES):
        m = {"x": np.ascontiguousarray(x[b]), "v": vrow}
        if has_bias:
            m["bias"] = np.ascontiguousarray(bias.reshape(1, S), dtype=np.float32)
        in_maps.append(m)

    res = run_bass_kernel_spmd(
        nc, in_maps, core_ids=list(range(N_CORES)), trace=trace
    )
    out = np.stack([r["out"] for r in res.results], axis=0)
    return out, res


def kernel(x, weight, bias, decay_value):
    out, _ = _run(x, weight, bias, decay_value)
    return out
